# revision 1
# baseline (speedup 1.0000x reference)
"""Trainium2 Bass kernel for Llama-like attention (16 heads, tanh softcap, RoPE).

Sharding: tensor-parallel over heads. Each of the 8 cores computes 2 heads:
  - q/k/v projections with column-sliced weights (x^T resident in SBUF,
    weight-slice streamed): q/k/v in natural [s, d] layout, f32 PSUM.
  - RoPE applied in natural layout. Weight columns of wq/wk are pre-permuted
    on the host to de-interleave even/odd rotary pairs, so rope works on
    contiguous 64-wide slices (the d-permutation cancels inside q.k).
  - attention with scores computed transposed ([kj, qi]) so the softmaxed
    probabilities feed the PV matmul directly as the moving operand.
    tanh softcap bounds scores, so softmax needs no row-max pass:
    p = exp(50*tanh(qk/(50*sqrt(hd)))), l = ones-row matmul, o = p@v / l.
  - per-head AllGather of o^T across cores, then each core contracts the
    full gathered o^T with its 256-column slice of wo and returns the
    transposed output slice; the host reassembles and transposes.
"""

import os
import sys

for _p in ("/root/.axon_site/_ro/trn_rl_repo", "/opt/trn_rl_repo"):
    if os.path.isdir(_p) and _p not in sys.path:
        sys.path.append(_p)

import numpy as np
import ml_dtypes
from contextlib import ExitStack

import concourse.bass as bass
import concourse.bacc as bacc
import concourse.mybir as mybir
import concourse.tile as tile
from concourse.bass_utils import run_bass_kernel_spmd
from concourse.masks import make_identity

BF16 = mybir.dt.bfloat16
F32 = mybir.dt.float32
NPBF16 = ml_dtypes.bfloat16

N_CORES = 8
S = 2048          # sequence length
DM = 2048         # model dim
H = 16            # heads
HD = 128          # head dim
HPC = H // N_CORES  # heads per core = 2
CW = HPC * HD     # per-core projection width = 256
P = 128
QT = 512          # query tile (free dim of attention matmuls)
NQT = S // QT     # 4 query tiles per head
NSC = S // P      # 16 sequence chunks
NKC = DM // P     # 16 contraction chunks
SOFTCAP = 50.0
C1 = 1.0 / (SOFTCAP * np.sqrt(HD))

Tanh = mybir.ActivationFunctionType.Tanh
Exp = mybir.ActivationFunctionType.Exp


def build_nc(reps=1, single=False):
    nc = bacc.Bacc("TRN2", target_bir_lowering=False, num_devices=N_CORES)

    xT_d = nc.dram_tensor("xT", [DM, S], BF16, kind="ExternalInput")
    w_d = nc.dram_tensor("w_all", [DM, 3 * CW], BF16, kind="ExternalInput")
    wo_d = nc.dram_tensor("wo_c", [DM, CW], BF16, kind="ExternalInput")
    cos_d = nc.dram_tensor("cos_b", [S, HD // 2], BF16, kind="ExternalInput")
    sin_d = nc.dram_tensor("sin_b", [S, HD // 2], BF16, kind="ExternalInput")
    mask_d = nc.dram_tensor("mask", [P, 4 * QT], BF16, kind="ExternalInput")
    out_d = nc.dram_tensor("outT", [CW, S], F32, kind="ExternalOutput")

    # collective bounce buffers (one per local head)
    ob = [nc.dram_tensor(f"ob{j}", [P, S], BF16) for j in range(HPC)]
    og = [
        nc.dram_tensor(f"og{j}", [N_CORES * P, S], BF16, addr_space="Shared")
        for j in range(HPC)
    ]

    with tile.TileContext(nc) as tc:
        for _rep in range(reps):
            _emit_body(nc, tc, xT_d, w_d, wo_d, cos_d, sin_d, mask_d, out_d,
                       ob, og, single)
    nc.compile()
    return nc


def _emit_body(nc, tc, xT_d, w_d, wo_d, cos_d, sin_d, mask_d, out_d, ob, og,
               single):
        with ExitStack() as ctx:
            # ---------- persistent SBUF ----------
            persist = ctx.enter_context(tc.tile_pool(name="persist", bufs=1))
            qT = [persist.tile([P, S], BF16, name=f"qT{h}") for h in range(HPC)]
            kT = [persist.tile([P, S], BF16, name=f"kT{h}") for h in range(HPC)]
            v_sb = [persist.tile([P, S], BF16, name=f"v{h}") for h in range(HPC)]
            oT = [persist.tile([P, S], BF16, name=f"oT{h}") for h in range(HPC)]
            mask_sb = persist.tile([P, 4 * QT], BF16, name="mask")
            ident = persist.tile([P, P], BF16, name="ident")
            ones_bf = persist.tile([P, 1], BF16, name="ones")
            cos_sb = persist.tile([P, NSC, HD // 2], BF16, name="cos")
            sin_sb = persist.tile([P, NSC, HD // 2], BF16, name="sin")

            nc.sync.dma_start(out=mask_sb[:], in_=mask_d[:])
            make_identity(nc, ident[:])
            nc.vector.memset(ones_bf[:], 1.0)
            # cos/sin: [S, 32] viewed as [NSC, P, 32] -> [P, NSC, 32]
            cos_r = cos_d.rearrange("(n p) f -> n p f", p=P)
            sin_r = sin_d.rearrange("(n p) f -> n p f", p=P)
            for i in range(NSC):
                nc.sync.dma_start(out=cos_sb[:, i, :], in_=cos_r[i])
                nc.sync.dma_start(out=sin_sb[:, i, :], in_=sin_r[i])

            # ---------- phase A: qkv projections + rope + transpose ----------
            with ExitStack() as ctxA:
                xp = ctxA.enter_context(tc.tile_pool(name="xT", bufs=1))
                wp = ctxA.enter_context(tc.tile_pool(name="w", bufs=1))
                rp = ctxA.enter_context(tc.tile_pool(name="rope", bufs=3))
                tmp = ctxA.enter_context(tc.tile_pool(name="ropetmp", bufs=4))
                qkv_ps = ctxA.enter_context(
                    tc.tile_pool(name="qkv_ps", bufs=2, space="PSUM")
                )
                tp_ps = ctxA.enter_context(
                    tc.tile_pool(name="tp_ps", bufs=2, space="PSUM")
                )

                # x^T split into 4 column groups so the first s-chunk's
                # matmuls only wait on the first quarter of the load
                NXQ = int(os.environ.get('KQ_NXQ', '4'))
                XQW = S // NXQ
                xt = [[xp.tile([P, XQW], BF16, name=f"xt{k}_{q}")
                       for q in range(NXQ)] for k in range(NKC)]
                wt = [wp.tile([P, 3 * CW], BF16, name=f"wt{k}") for k in range(NKC)]
                for k in range(NKC):
                    nc.sync.dma_start(out=wt[k][:], in_=w_d[k * P:(k + 1) * P, :])
                for q in range(NXQ):
                    for k in range(NKC):
                        nc.sync.dma_start(
                            out=xt[k][q][:],
                            in_=xT_d[k * P:(k + 1) * P, q * XQW:(q + 1) * XQW])

                HW = HD // 2  # 64
                for sc in range(NSC):
                    ps = qkv_ps.tile([P, 3 * CW], F32, name="qkv")
                    for k in range(NKC):
                        cpg = NSC // NXQ
                        lhsT = xt[k][sc // cpg][:, (sc % cpg) * P:(sc % cpg + 1) * P]
                        nc.tensor.matmul(
                            ps[:, 0:512], lhsT, wt[k][:, 0:512],
                            start=(k == 0), stop=(k == NKC - 1),
                        )
                        nc.tensor.matmul(
                            ps[:, 512:768], lhsT, wt[k][:, 512:768],
                            start=(k == 0), stop=(k == NKC - 1),
                        )
                    q_sb = rp.tile([P, CW], BF16, name="q_sb")
                    k_sb = rp.tile([P, CW], BF16, name="k_sb")
                    nc.scalar.copy(q_sb[:], ps[:, 0:CW])
                    nc.scalar.copy(k_sb[:], ps[:, CW:2 * CW])
                    for h in range(HPC):
                        nc.vector.tensor_copy(
                            v_sb[h][:, sc * P:(sc + 1) * P],
                            ps[:, 2 * CW + h * HD:2 * CW + (h + 1) * HD],
                        )
                    c_ap = cos_sb[:, sc, :]
                    s_ap = sin_sb[:, sc, :]
                    for src, rotT in ((q_sb, qT), (k_sb, kT)):
                        rot = rp.tile([P, CW], BF16, name="rot")
                        for h in range(HPC):
                            x0 = src[:, h * HD:h * HD + HW]
                            x1 = src[:, h * HD + HW:(h + 1) * HD]
                            t1 = tmp.tile([P, HW], BF16, name="t1")
                            t2 = tmp.tile([P, HW], BF16, name="t2")
                            nc.vector.tensor_mul(t1[:], x0, c_ap)
                            nc.vector.tensor_mul(t2[:], x1, s_ap)
                            nc.vector.tensor_sub(
                                rot[:, h * HD:h * HD + HW], t1[:], t2[:])
                            t3 = tmp.tile([P, HW], BF16, name="t3")
                            t4 = tmp.tile([P, HW], BF16, name="t4")
                            nc.vector.tensor_mul(t3[:], x0, s_ap)
                            nc.vector.tensor_mul(t4[:], x1, c_ap)
                            nc.vector.tensor_add(
                                rot[:, h * HD + HW:(h + 1) * HD], t3[:], t4[:])
                        for h in range(HPC):
                            tp = tp_ps.tile([P, P], BF16, name="tp")
                            nc.tensor.transpose(
                                tp[:], rot[:, h * HD:(h + 1) * HD], ident[:])
                            nc.vector.tensor_copy(
                                rotT[h][:, sc * P:(sc + 1) * P], tp[:])

            # ---------- phase B: attention ----------
            wop = ctx.enter_context(tc.tile_pool(name="wo", bufs=1))
            og0p = ctx.enter_context(tc.tile_pool(name="og0", bufs=4 * N_CORES))
            wo_sb = [wop.tile([P, CW], BF16, name=f"wo{i}") for i in range(NKC)]
            for i in range(NKC):
                nc.sync.dma_start(out=wo_sb[i][:], in_=wo_d[i * P:(i + 1) * P, :])
            og0t = {}
            with ExitStack() as ctxB:
                s_ps = ctxB.enter_context(
                    tc.tile_pool(name="s_ps", bufs=2, space="PSUM"))
                o_ps = ctxB.enter_context(
                    tc.tile_pool(name="o_ps", bufs=2, space="PSUM"))
                l_ps = ctxB.enter_context(
                    tc.tile_pool(name="l_ps", bufs=2, space="PSUM"))
                thp = ctxB.enter_context(tc.tile_pool(name="tanh", bufs=3))
                pp = ctxB.enter_context(tc.tile_pool(name="pT", bufs=3))
                np_ = ctxB.enter_context(tc.tile_pool(name="norm", bufs=2))

                for h in range(HPC):
                    for t in range(NQT):
                        o_acc = o_ps.tile([P, QT], F32, name="o_acc")
                        l_acc = l_ps.tile([1, QT], F32, name="l_acc")
                        npair = 2 * t + 2
                        q_ap = qT[h][:, t * QT:(t + 1) * QT]

                        def emit_pv(pT, p, last):
                            for i in range(2):
                                kc = 2 * p + i
                                nc.tensor.matmul(
                                    o_acc[:],
                                    v_sb[h][:, kc * P:(kc + 1) * P],
                                    pT[:, i * QT:(i + 1) * QT],
                                    start=(kc == 0), stop=(last and i == 1),
                                )
                                nc.tensor.matmul(
                                    l_acc[:], ones_bf[:, 0:1],
                                    pT[:, i * QT:(i + 1) * QT],
                                    start=(kc == 0), stop=(last and i == 1),
                                )

                        prev = None
                        for p in range(npair):
                            # scores for chunk pair (2p, 2p+1), each [P, QT]
                            sp = s_ps.tile([P, 2 * QT], F32, name="sp")
                            for i in range(2):
                                kc = 2 * p + i
                                nc.tensor.matmul(
                                    sp[:, i * QT:(i + 1) * QT],
                                    kT[h][:, kc * P:(kc + 1) * P], q_ap,
                                    start=True, stop=True,
                                )
                            th = thp.tile([P, 2 * QT], F32, name="th")
                            nc.scalar.activation(th[:], sp[:], Tanh, scale=C1)
                            pT = pp.tile([P, 2 * QT], BF16, name="pTt")
                            nc.scalar.activation(pT[:], th[:], Exp, scale=SOFTCAP)
                            # masked pairs are the last two: p==2t (u=0,1) and
                            # p==2t+1 (u=2,3); mask_sb columns line up 1:1
                            u0 = 2 * (p - 2 * t)
                            if u0 >= 0:
                                nc.vector.tensor_mul(
                                    pT[:], pT[:],
                                    mask_sb[:, u0 * QT:(u0 + 2) * QT])
                            if prev is not None:
                                emit_pv(prev[0], prev[1], last=False)
                            prev = (pT, p)
                        emit_pv(prev[0], prev[1], last=True)
                        recip = np_.tile([1, QT], F32, name="recip")
                        nc.vector.reciprocal(recip[:], l_acc[:])
                        bcast = np_.tile([P, QT], F32, name="bcast")
                        nc.gpsimd.partition_broadcast(bcast[:], recip[:])
                        nc.vector.tensor_mul(
                            oT[h][:, t * QT:(t + 1) * QT], o_acc[:], bcast[:])
                    nc.sync.dma_start(out=ob[h][:], in_=oT[h][:])
                    if single:
                        # timeline-sim stand-in for the AllGather
                        nc.gpsimd.dma_start(out=og[h][0:P, :], in_=ob[h][:])
                    else:
                        nc.gpsimd.collective_compute(
                            "AllGather", mybir.AluOpType.bypass,
                            replica_groups=[list(range(N_CORES))],
                            ins=[ob[h][:]], outs=[og[h][:]],
                        )
                    if h == 0:
                        # stream gathered head-0 o^T into SBUF during
                        # head 1's (ACT-bound) attention
                        for n in range(NQT):
                            for k in range(N_CORES):
                                t_ = og0p.tile([P, QT], BF16, name="og0t")
                                nc.sync.dma_start(
                                    out=t_[:],
                                    in_=og[0][k * P:(k + 1) * P,
                                              n * QT:(n + 1) * QT],
                                )
                                og0t[n, k] = t_

            # ---------- phase C: output projection ----------
            # The og0 half of the contraction is emitted first so it runs on
            # PE while the second AllGather is still in flight; og1 closes
            # the accumulation groups.
            with ExitStack() as ctxC:
                og1p = ctxC.enter_context(tc.tile_pool(name="og1", bufs=4 * N_CORES))
                outp = ctxC.enter_context(tc.tile_pool(name="out", bufs=3))
                wo_ps = ctxC.enter_context(
                    tc.tile_pool(name="wo_ps", bufs=NQT * HPC, space="PSUM"))

                accs = {}
                for n in range(NQT):
                    for m in range(HPC):
                        acc = wo_ps.tile([P, QT], F32, name="acc")
                        accs[n, m] = acc
                        for k in range(N_CORES):
                            nc.tensor.matmul(
                                acc[:], wo_sb[k][:, m * P:(m + 1) * P],
                                og0t[n, k][:],
                                start=(k == 0), stop=False,
                            )
                og1t = {}
                for n in range(NQT):
                    for k in range(N_CORES):
                        t_ = og1p.tile([P, QT], BF16, name="og1t")
                        nc.sync.dma_start(
                            out=t_[:],
                            in_=og[1][k * P:(k + 1) * P, n * QT:(n + 1) * QT],
                        )
                        og1t[n, k] = t_
                for n in range(NQT):
                    for m in range(HPC):
                        acc = accs[n, m]
                        for k in range(N_CORES):
                            nc.tensor.matmul(
                                acc[:], wo_sb[N_CORES + k][:, m * P:(m + 1) * P],
                                og1t[n, k][:],
                                start=False, stop=(k == N_CORES - 1),
                            )
                        osb = outp.tile([P, QT], F32, name="osb")
                        nc.scalar.copy(osb[:], acc[:])
                        nc.sync.dma_start(
                            out=out_d[m * P:(m + 1) * P, n * QT:(n + 1) * QT],
                            in_=osb[:],
                        )


_NC_CACHE = None


def _get_nc():
    global _NC_CACHE
    if _NC_CACHE is None:
        _NC_CACHE = build_nc()
    return _NC_CACHE


def _rope_perm():
    """per-head column permutation de-interleaving rotary pairs"""
    perm = np.zeros(DM, np.int64)
    for h in range(H):
        base = h * HD
        perm[base:base + HD // 2] = base + np.arange(0, HD, 2)
        perm[base + HD // 2:base + HD] = base + np.arange(1, HD, 2)
    return perm


def make_in_maps(x, wq, wk, wv, wo, freqs_cos, freqs_sin):
    x = np.asarray(x, np.float32).reshape(S, DM)
    wq = np.asarray(wq, np.float32)
    wk = np.asarray(wk, np.float32)
    wv = np.asarray(wv, np.float32)
    wo = np.asarray(wo, np.float32)
    xT = np.ascontiguousarray(x.T).astype(NPBF16)
    perm = _rope_perm()
    wq_p = wq[:, perm]
    wk_p = wk[:, perm]
    cos_b = np.asarray(freqs_cos, np.float32).astype(NPBF16)
    sin_b = np.asarray(freqs_sin, np.float32).astype(NPBF16)
    # mask[i, u*QT + j] = 1 if i <= j - 128*u else 0  (keep kj <= qi)
    i_idx = np.arange(P)[:, None]
    j_idx = np.arange(QT)[None, :]
    mask = np.concatenate(
        [(i_idx <= j_idx - P * u) for u in range(4)], axis=1
    ).astype(NPBF16)
    # wo rows reordered to match AllGather row order: og[j] rows are
    # (core r, local head j) = global head 2r+j
    wo_r = np.concatenate(
        [
            np.concatenate(
                [wo[(HPC * r + j) * HD:(HPC * r + j + 1) * HD, :]
                 for r in range(N_CORES)], axis=0)
            for j in range(HPC)
        ],
        axis=0,
    )
    in_maps = []
    for c in range(N_CORES):
        cs = slice(c * CW, (c + 1) * CW)
        w_all = np.concatenate(
            [wq_p[:, cs], wk_p[:, cs], wv[:, cs]], axis=1).astype(NPBF16)
        wo_c = np.ascontiguousarray(wo_r[:, cs]).astype(NPBF16)
        in_maps.append({
            "xT": xT,
            "w_all": np.ascontiguousarray(w_all),
            "wo_c": wo_c,
            "cos_b": cos_b,
            "sin_b": sin_b,
            "mask": mask,
        })
    return in_maps


def assemble_output(results):
    outT = np.concatenate([r["outT"] for r in results], axis=0)  # [DM, S]
    return np.ascontiguousarray(outT.T).reshape(1, S, DM).astype(np.float32)


def kernel(x, wq, wk, wv, wo, freqs_cos, freqs_sin):
    nc = _get_nc()
    in_maps = make_in_maps(x, wq, wk, wv, wo, freqs_cos, freqs_sin)
    res = run_bass_kernel_spmd(nc, in_maps, core_ids=list(range(N_CORES)))
    return assemble_output(res.results)


if __name__ == "__main__":
    rng = np.random.default_rng(0)
    ins = {
        "x": rng.standard_normal((1, S, DM), np.float32),
        "wq": rng.standard_normal((DM, DM), np.float32) / np.sqrt(DM),
        "wk": rng.standard_normal((DM, DM), np.float32) / np.sqrt(DM),
        "wv": rng.standard_normal((DM, DM), np.float32) / np.sqrt(DM),
        "wo": rng.standard_normal((DM, DM), np.float32) / np.sqrt(DM),
        "freqs_cos": rng.standard_normal((S, HD // 2), np.float32),
        "freqs_sin": rng.standard_normal((S, HD // 2), np.float32),
    }
    out = kernel(**ins)
    print("out", out.shape, out.dtype, np.abs(out).mean())



# revision 6
# speedup vs baseline: 1.0672x; 1.0672x over previous
"""Trainium2 Bass kernel for Llama-like attention (16 heads, tanh softcap, RoPE).

Sharding: tensor-parallel over heads, fully collective-free. Each of the 8
cores computes 2 heads end-to-end and a *partial* output projection
(o_local @ wo_rows_local)^T; the host sums the 8 partial outputs. With no
on-device collective, each core's NEFF span is pure local compute — no
cross-core rendezvous.

Per-core pipeline:
  - q^T/k^T computed directly in transposed layout ([head_dim, s]) via
    matmul(w_slice^T, x^T), so no PE transposes are needed. Weight columns
    of wq/wk are pre-permuted on the host to de-interleave even/odd rotary
    pairs (the permutation cancels inside q.k).
  - RoPE applied in transposed layout with transposed cos/sin tables:
    rot = A*C + swap(A)*S' where C = [cosT; cosT], S' = [-sinT; sinT].
    Swap copies run on ACT, multiplies/adds on Pool (gpsimd), keeping the
    Vector engine free for the attention inner loop.
  - attention with scores computed transposed ([kj, qi]) so the softmaxed
    probabilities feed the PV matmul directly as the moving operand.
    tanh softcap bounds scores, so softmax needs no row-max pass:
    p = exp(50*tanh(qk/(50*sqrt(hd)))), l = ones-row matmul, o = p@v / l.
  - output projection: acc[oc, s] += wo_h[:, oc]^T @ oT_h accumulated over
    the 2 local heads in 4-bank PSUM tiles, DMA'd straight from PSUM to
    DRAM (f32). Host sums partials across cores and transposes.
"""

import os
import sys

for _p in ("/root/.axon_site/_ro/trn_rl_repo", "/opt/trn_rl_repo"):
    if os.path.isdir(_p) and _p not in sys.path:
        sys.path.append(_p)

import numpy as np
import ml_dtypes
from contextlib import ExitStack

import concourse.bass as bass
import concourse.bacc as bacc
import concourse.mybir as mybir
import concourse.tile as tile
from concourse.bass_utils import run_bass_kernel_spmd

BF16 = mybir.dt.bfloat16
F32 = mybir.dt.float32
NPBF16 = ml_dtypes.bfloat16

N_CORES = 8
S = 2048          # sequence length
DM = 2048         # model dim
H = 16            # heads
HD = 128          # head dim
HPC = H // N_CORES  # heads per core = 2
CW = HPC * HD     # per-core projection width = 256
P = 128
QT = 512          # query tile (free dim of attention matmuls)
NQT = S // QT     # 4 query tiles per head
NSC = S // P      # 16 sequence chunks
NKC = DM // P     # 16 contraction chunks
NST = S // QT     # 4 s-tiles
SOFTCAP = 50.0
C1 = 1.0 / (SOFTCAP * np.sqrt(HD))

Tanh = mybir.ActivationFunctionType.Tanh
Exp = mybir.ActivationFunctionType.Exp


def build_nc(reps=1, single=False):
    nc = bacc.Bacc("TRN2", target_bir_lowering=False, num_devices=N_CORES)

    xT_d = nc.dram_tensor("xT", [DM, S], BF16, kind="ExternalInput")
    w_d = nc.dram_tensor("w_all", [DM, 3 * CW], BF16, kind="ExternalInput")
    wo_d = nc.dram_tensor("wo_c", [CW, DM], BF16, kind="ExternalInput")
    cos_d = nc.dram_tensor("cosT2", [P, S], BF16, kind="ExternalInput")
    sin_d = nc.dram_tensor("sinT2", [P, S], BF16, kind="ExternalInput")
    mask_d = nc.dram_tensor("mask", [P, 4 * QT], BF16, kind="ExternalInput")
    out_d = nc.dram_tensor("outT", [DM, S], BF16, kind="ExternalOutput")

    with tile.TileContext(nc) as tc:
        for _rep in range(reps):
            _emit_body(nc, tc, xT_d, w_d, wo_d, cos_d, sin_d, mask_d, out_d)
    nc.compile()
    return nc


def _emit_body(nc, tc, xT_d, w_d, wo_d, cos_d, sin_d, mask_d, out_d):
    with ExitStack() as ctx:
        # ---------- persistent SBUF ----------
        persist = ctx.enter_context(tc.tile_pool(name="persist", bufs=1))
        qT = [persist.tile([P, S], BF16, name=f"qT{h}") for h in range(HPC)]
        kT = [persist.tile([P, S], BF16, name=f"kT{h}") for h in range(HPC)]
        v_sb = [persist.tile([P, S], BF16, name=f"v{h}") for h in range(HPC)]
        oT = [persist.tile([P, S], BF16, name=f"oT{h}") for h in range(HPC)]
        mask_sb = persist.tile([P, 4 * QT], BF16, name="mask")
        ones_bf = persist.tile([P, 1], BF16, name="ones")
        cos_sb = persist.tile([P, S], BF16, name="cosT2")
        sin_sb = persist.tile([P, S], BF16, name="sinT2")
        wo_sb = [persist.tile([P, DM], BF16, name=f"wo{h}") for h in range(HPC)]

        nc.sync.dma_start(out=mask_sb[:], in_=mask_d[:])
        nc.vector.memset(ones_bf[:], 1.0)
        nc.sync.dma_start(out=cos_sb[:], in_=cos_d[:])
        nc.sync.dma_start(out=sin_sb[:], in_=sin_d[:])
        for h in range(HPC):
            nc.sync.dma_start(out=wo_sb[h][:], in_=wo_d[h * P:(h + 1) * P, :])

        # ---------- phase A: qkv projections + rope ----------
        with ExitStack() as ctxA:
            xp = ctxA.enter_context(tc.tile_pool(name="xT", bufs=1))
            wp = ctxA.enter_context(tc.tile_pool(name="w", bufs=1))
            rp = ctxA.enter_context(tc.tile_pool(name="rope", bufs=4))
            qk_ps = ctxA.enter_context(
                tc.tile_pool(name="qk_ps", bufs=3, space="PSUM")
            )
            v_ps = ctxA.enter_context(
                tc.tile_pool(name="v_ps", bufs=2, space="PSUM")
            )

            wt = [wp.tile([P, 3 * CW], BF16, name=f"wt{k}") for k in range(NKC)]
            for k in range(NKC):
                nc.sync.dma_start(out=wt[k][:], in_=w_d[k * P:(k + 1) * P, :])
            xt = [xp.tile([P, S], BF16, name=f"xt{k}") for k in range(NKC)]
            for k in range(NKC):
                nc.sync.dma_start(out=xt[k][:], in_=xT_d[k * P:(k + 1) * P, :])

            # q/k in transposed layout: psum[d(128), s(512)] for each of the
            # 4 feature chunks (q-h0, q-h1, k-h0, k-h1) x 4 s-tiles.
            # Chunk c covers w_all columns [c*128, (c+1)*128).
            HW = HD // 2  # 64
            for c in range(4):
                dst = (qT, kT)[c // HPC][c % HPC]
                for st in range(NST):
                    ps = qk_ps.tile([P, QT], F32, name="qk")
                    for k in range(NKC):
                        nc.tensor.matmul(
                            ps[:],
                            wt[k][:, c * P:(c + 1) * P],
                            xt[k][:, st * QT:(st + 1) * QT],
                            start=(k == 0), stop=(k == NKC - 1),
                        )
                    # rope: rows 0:64 = x0, 64:128 = x1 (host de-interleave)
                    a_sb = rp.tile([P, QT], BF16, name="a_sb")
                    a_sw = rp.tile([P, QT], BF16, name="a_sw")
                    nc.scalar.copy(a_sb[:], ps[:])
                    nc.scalar.copy(a_sw[0:HW, :], ps[HW:HD, :])
                    nc.scalar.copy(a_sw[HW:HD, :], ps[0:HW, :])
                    t1 = rp.tile([P, QT], BF16, name="t1")
                    cs = slice(st * QT, (st + 1) * QT)
                    nc.gpsimd.tensor_mul(t1[:], a_sb[:], cos_sb[:, cs])
                    t2 = rp.tile([P, QT], BF16, name="t2")
                    nc.gpsimd.tensor_mul(t2[:], a_sw[:], sin_sb[:, cs])
                    nc.gpsimd.tensor_add(dst[:, cs], t1[:], t2[:])

            # v in natural layout: psum[s(128), 2*HD] per s-chunk
            for sc in range(NSC):
                ps = v_ps.tile([P, CW], F32, name="v")
                for k in range(NKC):
                    nc.tensor.matmul(
                        ps[:],
                        xt[k][:, sc * P:(sc + 1) * P],
                        wt[k][:, 2 * CW:3 * CW],
                        start=(k == 0), stop=(k == NKC - 1),
                    )
                for h in range(HPC):
                    nc.vector.tensor_copy(
                        v_sb[h][:, sc * P:(sc + 1) * P],
                        ps[:, h * HD:(h + 1) * HD],
                    )

        # ---------- phase B: attention ----------
        with ExitStack() as ctxB:
            s_ps = ctxB.enter_context(
                tc.tile_pool(name="s_ps", bufs=2, space="PSUM"))
            o_ps = ctxB.enter_context(
                tc.tile_pool(name="o_ps", bufs=2, space="PSUM"))
            l_ps = ctxB.enter_context(
                tc.tile_pool(name="l_ps", bufs=2, space="PSUM"))
            thp = ctxB.enter_context(tc.tile_pool(name="tanh", bufs=3))
            pp = ctxB.enter_context(tc.tile_pool(name="pT", bufs=3))
            np_ = ctxB.enter_context(tc.tile_pool(name="norm", bufs=2))

            for h in range(HPC):
                for t in range(NQT):
                    o_acc = o_ps.tile([P, QT], F32, name="o_acc")
                    l_acc = l_ps.tile([1, QT], F32, name="l_acc")
                    npair = 2 * t + 2
                    q_ap = qT[h][:, t * QT:(t + 1) * QT]

                    def emit_pv(pT, p, last):
                        for i in range(2):
                            kc = 2 * p + i
                            nc.tensor.matmul(
                                o_acc[:],
                                v_sb[h][:, kc * P:(kc + 1) * P],
                                pT[:, i * QT:(i + 1) * QT],
                                start=(kc == 0), stop=(last and i == 1),
                            )
                            nc.tensor.matmul(
                                l_acc[:], ones_bf[:, 0:1],
                                pT[:, i * QT:(i + 1) * QT],
                                start=(kc == 0), stop=(last and i == 1),
                            )

                    prev = None
                    for p in range(npair):
                        # scores for chunk pair (2p, 2p+1), each [P, QT]
                        sp = s_ps.tile([P, 2 * QT], F32, name="sp")
                        for i in range(2):
                            kc = 2 * p + i
                            nc.tensor.matmul(
                                sp[:, i * QT:(i + 1) * QT],
                                kT[h][:, kc * P:(kc + 1) * P], q_ap,
                                start=True, stop=True,
                            )
                        th = thp.tile([P, 2 * QT], F32, name="th")
                        nc.scalar.activation(th[:], sp[:], Tanh, scale=C1)
                        pT = pp.tile([P, 2 * QT], BF16, name="pTt")
                        nc.scalar.activation(pT[:], th[:], Exp, scale=SOFTCAP)
                        # masked pairs are the last two: p==2t (u=0,1) and
                        # p==2t+1 (u=2,3); mask_sb columns line up 1:1
                        u0 = 2 * (p - 2 * t)
                        if u0 >= 0:
                            nc.vector.tensor_mul(
                                pT[:], pT[:],
                                mask_sb[:, u0 * QT:(u0 + 2) * QT])
                        if prev is not None:
                            emit_pv(prev[0], prev[1], last=False)
                        prev = (pT, p)
                    emit_pv(prev[0], prev[1], last=True)
                    recip = np_.tile([1, QT], F32, name="recip")
                    nc.vector.reciprocal(recip[:], l_acc[:])
                    bcast = np_.tile([P, QT], F32, name="bcast")
                    nc.gpsimd.partition_broadcast(bcast[:], recip[:])
                    nc.vector.tensor_mul(
                        oT[h][:, t * QT:(t + 1) * QT], o_acc[:], bcast[:])

        # ---------- phase C: partial output projection ----------
        # acc[oc(128), s(2048)] = sum_h wo_h[:, oc]^T @ oT_h, DMA'd straight
        # from PSUM (4 banks per tile, double-buffered).
        with ExitStack() as ctxC:
            wo_ps = ctxC.enter_context(
                tc.tile_pool(name="wo_ps", bufs=2, space="PSUM"))
            outp = ctxC.enter_context(tc.tile_pool(name="out", bufs=3))
            for oc in range(NKC):
                acc = wo_ps.tile([P, S], F32, name="acc")
                for st in range(NST):
                    for h in range(HPC):
                        nc.tensor.matmul(
                            acc[:, st * QT:(st + 1) * QT],
                            wo_sb[h][:, oc * P:(oc + 1) * P],
                            oT[h][:, st * QT:(st + 1) * QT],
                            start=(h == 0), stop=(h == HPC - 1),
                        )
                osb = outp.tile([P, S], BF16, name="osb")
                if oc % 2 == 0:
                    nc.scalar.copy(osb[:], acc[:])
                else:
                    nc.vector.tensor_copy(osb[:], acc[:])
                nc.sync.dma_start(
                    out=out_d[oc * P:(oc + 1) * P, :], in_=osb[:])


_NC_CACHE = None


def _get_nc():
    global _NC_CACHE
    if _NC_CACHE is None:
        _NC_CACHE = build_nc()
    return _NC_CACHE


def _rope_perm():
    """per-head column permutation de-interleaving rotary pairs"""
    perm = np.zeros(DM, np.int64)
    for h in range(H):
        base = h * HD
        perm[base:base + HD // 2] = base + np.arange(0, HD, 2)
        perm[base + HD // 2:base + HD] = base + np.arange(1, HD, 2)
    return perm


def make_in_maps(x, wq, wk, wv, wo, freqs_cos, freqs_sin):
    x = np.asarray(x, np.float32).reshape(S, DM)
    wq = np.asarray(wq, np.float32)
    wk = np.asarray(wk, np.float32)
    wv = np.asarray(wv, np.float32)
    wo = np.asarray(wo, np.float32)
    xT = np.ascontiguousarray(x.T).astype(NPBF16)
    perm = _rope_perm()
    wq_p = wq[:, perm]
    wk_p = wk[:, perm]
    # transposed rope tables: C = [cosT; cosT], S' = [-sinT; sinT]
    cosT = np.asarray(freqs_cos, np.float32).T  # [64, S]
    sinT = np.asarray(freqs_sin, np.float32).T
    cosT2 = np.concatenate([cosT, cosT], axis=0).astype(NPBF16)
    sinT2 = np.concatenate([-sinT, sinT], axis=0).astype(NPBF16)
    # mask[i, u*QT + j] = 1 if i <= j - 128*u else 0  (keep kj <= qi)
    i_idx = np.arange(P)[:, None]
    j_idx = np.arange(QT)[None, :]
    mask = np.concatenate(
        [(i_idx <= j_idx - P * u) for u in range(4)], axis=1
    ).astype(NPBF16)
    in_maps = []
    for c in range(N_CORES):
        cs = slice(c * CW, (c + 1) * CW)
        w_all = np.concatenate(
            [wq_p[:, cs], wk_p[:, cs], wv[:, cs]], axis=1).astype(NPBF16)
        wo_c = np.ascontiguousarray(wo[cs, :]).astype(NPBF16)
        in_maps.append({
            "xT": xT,
            "w_all": np.ascontiguousarray(w_all),
            "wo_c": wo_c,
            "cosT2": cosT2,
            "sinT2": sinT2,
            "mask": mask,
        })
    return in_maps


def assemble_output(results):
    acc = results[0]["outT"].astype(np.float32)
    for r in results[1:]:
        acc += np.asarray(r["outT"]).astype(np.float32)
    return np.ascontiguousarray(acc.T).reshape(1, S, DM).astype(np.float32)


def kernel(x, wq, wk, wv, wo, freqs_cos, freqs_sin):
    nc = _get_nc()
    in_maps = make_in_maps(x, wq, wk, wv, wo, freqs_cos, freqs_sin)
    res = run_bass_kernel_spmd(nc, in_maps, core_ids=list(range(N_CORES)))
    return assemble_output(res.results)


if __name__ == "__main__":
    rng = np.random.default_rng(0)
    ins = {
        "x": rng.standard_normal((1, S, DM), np.float32),
        "wq": rng.standard_normal((DM, DM), np.float32) / np.sqrt(DM),
        "wk": rng.standard_normal((DM, DM), np.float32) / np.sqrt(DM),
        "wv": rng.standard_normal((DM, DM), np.float32) / np.sqrt(DM),
        "wo": rng.standard_normal((DM, DM), np.float32) / np.sqrt(DM),
        "freqs_cos": rng.standard_normal((S, HD // 2), np.float32),
        "freqs_sin": rng.standard_normal((S, HD // 2), np.float32),
    }
    out = kernel(**ins)
    print("out", out.shape, out.dtype, np.abs(out).mean())


# revision 28
# speedup vs baseline: 1.2472x; 1.1687x over previous
"""Trainium2 Bass kernel for Llama-like attention (16 heads, tanh softcap, RoPE).

Sharding: tensor-parallel over heads, fully collective-free. Each of the 8
cores computes 2 heads end-to-end and a *partial* output projection
(o_local @ wo_rows_local)^T; the host sums the 8 partial outputs. With no
on-device collective, each core's NEFF span is pure local compute — no
cross-core rendezvous.

Per-core pipeline (engine-balanced against the ~165us PE floor):
  - q^T/k^T computed directly in transposed layout ([head_dim, s]) via
    matmul(w_slice^T, x^T): no PE transposes. Weight columns of wq/wk are
    pre-permuted on the host to de-interleave even/odd rotary pairs (the
    permutation cancels inside q.k).
  - RoPE in transposed layout straight out of PSUM: rot = A*C + swap(A)*S'
    with C = [cosT; cosT], S' = [-sinT; sinT]. The partition-half swap is
    done by two half-height Pool multiplies reading PSUM at a partition
    offset; the straight multiply and the final add run on Vector (bf16
    2x mode). ACT stays free for the softmax chain.
  - attention with scores transposed ([kj, qi]) so softmaxed probabilities
    feed the PV matmul directly as the moving operand. tanh softcap bounds
    scores, so softmax needs no row-max pass: p = exp(50*tanh(.)),
    l = ones-row matmul, o = p@v / l. Head 0's ACT-bound window is filled
    with head 1's q/k projection and the tail v chunks; head 1's windows
    are filled with the output-projection pieces for the q-tile that just
    finished.
  - output projection pieces acc[oc(128), st(512)] += wo_h[:, oc]^T @ oT_h
    accumulated over the 2 local heads, copied to SBUF bf16 (ACT/DVE
    alternating) and DMA'd per piece. Host sums partials and transposes.
"""

import os
import sys

for _p in ("/root/.axon_site/_ro/trn_rl_repo", "/opt/trn_rl_repo"):
    if os.path.isdir(_p) and _p not in sys.path:
        sys.path.append(_p)

import numpy as np
import ml_dtypes
from contextlib import ExitStack

import concourse.bass as bass
import concourse.bacc as bacc
import concourse.mybir as mybir
import concourse.tile as tile
from concourse.bass_utils import run_bass_kernel_spmd

BF16 = mybir.dt.bfloat16
F32 = mybir.dt.float32
NPBF16 = ml_dtypes.bfloat16

N_CORES = 8
S = 2048          # sequence length
DM = 2048         # model dim
H = 16            # heads
HD = 128          # head dim
HPC = H // N_CORES  # heads per core = 2
CW = HPC * HD     # per-core projection width = 256
P = 128
HW = HD // 2      # 64
QT = 512          # query tile (free dim of attention matmuls)
NQT = S // QT     # 4 query tiles per head
NSC = S // P      # 16 sequence chunks
NKC = DM // P     # 16 contraction chunks
NST = S // QT     # 4 s-tiles
SOFTCAP = 50.0
C1 = 1.0 / (SOFTCAP * np.sqrt(HD))

Tanh = mybir.ActivationFunctionType.Tanh
Exp = mybir.ActivationFunctionType.Exp


def build_nc(reps=1, single=False):
    nc = bacc.Bacc("TRN2", target_bir_lowering=False, num_devices=N_CORES)

    xT_d = nc.dram_tensor("xT", [DM, S], BF16, kind="ExternalInput")
    w_d = nc.dram_tensor("w_all", [DM, 3 * CW], BF16, kind="ExternalInput")
    wo_d = nc.dram_tensor("wo_c", [CW, DM], BF16, kind="ExternalInput")
    cos_d = nc.dram_tensor("cosT2", [P, S], BF16, kind="ExternalInput")
    sin_d = nc.dram_tensor("sinT2", [P, S], BF16, kind="ExternalInput")
    mask_d = nc.dram_tensor("mask", [P, 4 * QT], BF16, kind="ExternalInput")
    out_d = nc.dram_tensor("outT", [DM, S], BF16, kind="ExternalOutput")

    with tile.TileContext(nc) as tc:
        for _rep in range(reps):
            _emit_body(nc, tc, xT_d, w_d, wo_d, cos_d, sin_d, mask_d, out_d)
    nc.compile()
    return nc


def _emit_body(nc, tc, xT_d, w_d, wo_d, cos_d, sin_d, mask_d, out_d):
    with ExitStack() as ctx:
        # ---------- persistent SBUF ----------
        persist = ctx.enter_context(tc.tile_pool(name="persist", bufs=1))
        qT = [persist.tile([P, S], BF16, name=f"qT{h}") for h in range(HPC)]
        kT = [persist.tile([P, S], BF16, name=f"kT{h}") for h in range(HPC)]
        v_sb = [persist.tile([P, S], BF16, name=f"v{h}") for h in range(HPC)]
        oT = [persist.tile([P, S], BF16, name=f"oT{h}") for h in range(HPC)]
        mask_sb = persist.tile([P, 4 * QT], BF16, name="mask")
        ones_bf = persist.tile([P, 1], BF16, name="ones")
        cos_sb = persist.tile([P, S], BF16, name="cosT2")
        sin_sb = persist.tile([P, S], BF16, name="sinT2")
        wo_sb = [persist.tile([P, DM], BF16, name=f"wo{h}") for h in range(HPC)]
        xp = ctx.enter_context(tc.tile_pool(name="xT", bufs=1))
        wp = ctx.enter_context(tc.tile_pool(name="w", bufs=1))
        rp = ctx.enter_context(tc.tile_pool(name="rope", bufs=4))

        # DMA priority order (HWDGE + the transfer engines serialize, so
        # issue order IS arrival order): rope tables, then the pre-phase
        # wavefront (w cols [q0|k0|v] + x first halves, k-interleaved),
        # then x second halves, mask, w cols [q1|k1], wo. w_all columns
        # are host-reordered to [q0, k0, v, q1, k1] to enable the split.
        # Batched loads: HWDGE issue bandwidth (~0.63us per DMA) is the
        # startup bottleneck, so w/x load as 4-k-group DMAs via 3-D tiles
        # and partition-inner DRAM views; x additionally splits into
        # column halves so the first s-tiles unblock early.
        wt_all = wp.tile([P, NKC, 3 * CW], BF16, name="wt")
        xt_all = xp.tile([P, NKC, S], BF16, name="xt")
        w_r = w_d.rearrange("(k p) c -> p k c", p=P)
        x_r = xT_d.rearrange("(k p) c -> p k c", p=P)
        HS = S // 2
        for g in range(0, NKC, 4):
            nc.sync.dma_start(
                out=wt_all[:, g:g + 4, :], in_=w_r[:, g:g + 4, :])
            nc.scalar.dma_start(
                out=xt_all[:, g:g + 4, 0:HS], in_=x_r[:, g:g + 4, 0:HS])
            if g == 0:
                # rope tables: needed ~10us in, after the first k-group
                nc.scalar.dma_start(out=cos_sb[:], in_=cos_d[:])
                nc.scalar.dma_start(out=sin_sb[:], in_=sin_d[:])
        for g in range(0, NKC, 4):
            (nc.sync if g % 8 == 0 else nc.scalar).dma_start(
                out=xt_all[:, g:g + 4, HS:S], in_=x_r[:, g:g + 4, HS:S])
        nc.sync.dma_start(out=mask_sb[:], in_=mask_d[:])
        nc.vector.memset(ones_bf[:], 1.0)
        for h in range(HPC):
            nc.sync.dma_start(out=wo_sb[h][:], in_=wo_d[h * P:(h + 1) * P, :])

        # w_all column offsets after host reorder [q0, k0, v, q1, k1]
        W_OFF = {0: 0, 2: P, 1: 2 * P + CW, 3: 3 * P + CW}
        V_OFF = 2 * P

        def qk_chunks(pool, c, st):
            """q/k feature chunk c (0: q-h0, 1: q-h1, 2: k-h0, 3: k-h1),
            s-tile st, transposed layout + fused rope, as 4 PE micro-steps."""
            dst = (qT, kT)[c // HPC][c % HPC]
            wo_ = W_OFF[c]
            state = {}

            def mm(k0):
                def f():
                    if k0 == 0:
                        state["ps"] = pool.tile([P, QT], F32, name="f")
                    ps = state["ps"]
                    for k in range(k0, k0 + 4):
                        nc.tensor.matmul(
                            ps[:],
                            wt_all[:, k, wo_:wo_ + P],
                            xt_all[:, k, st * QT:(st + 1) * QT],
                            start=(k == 0), stop=(k == NKC - 1),
                        )
                    if k0 == NKC - 4:
                        ps = state["ps"]
                        cs = slice(st * QT, (st + 1) * QT)
                        # Pool cannot read PSUM: both rope multiplies run
                        # on DVE; the all-SBUF add goes to Pool.
                        t1 = rp.tile([P, QT], BF16, name="t1")
                        nc.vector.tensor_mul(t1[:], ps[:], cos_sb[:, cs])
                        t2 = rp.tile([P, QT], BF16, name="t2")
                        nc.vector.tensor_mul(
                            t2[0:HW, :], ps[HW:HD, :], sin_sb[0:HW, cs])
                        nc.vector.tensor_mul(
                            t2[HW:HD, :], ps[0:HW, :], sin_sb[HW:HD, cs])
                        nc.gpsimd.tensor_add(dst[:, cs], t1[:], t2[:])
                return f
            return [mm(k0) for k0 in range(0, NKC, 4)]

        def v_chunks(pool, sc):
            """v s-chunk sc in natural layout, as 2 PE micro-steps."""
            state = {}

            def mm(k0):
                def f():
                    if k0 == 0:
                        state["ps"] = pool.tile([P, QT], F32, name="f")
                    ps = state["ps"]
                    for k in range(k0, k0 + 8):
                        nc.tensor.matmul(
                            ps[:, 0:CW],
                            xt_all[:, k, sc * P:(sc + 1) * P],
                            wt_all[:, k, V_OFF:V_OFF + CW],
                            start=(k == 0), stop=(k == NKC - 1),
                        )
                    if k0 == NKC - 8:
                        for h in range(HPC):
                            nc.vector.tensor_copy(
                                v_sb[h][:, sc * P:(sc + 1) * P],
                                ps[:, h * HD:(h + 1) * HD],
                            )
                return f
            return [mm(0), mm(8)]

        o_r = out_d.rearrange("(o p) s -> p o s", p=P)

        def c_chunks(st, outp, c_ps):
            """output-projection pieces for s-tile st, 1 PE micro-step each;
            results stage into 4-oc-wide tiles DMA'd as one transfer."""
            state = {}

            def piece(oc):
                def f():
                    acc = c_ps.tile([P, QT], F32, name="f")
                    for h in range(HPC):
                        nc.tensor.matmul(
                            acc[:],
                            wo_sb[h][:, oc * P:(oc + 1) * P],
                            oT[h][:, st * QT:(st + 1) * QT],
                            start=(h == 0), stop=(h == HPC - 1),
                        )
                    if oc % 4 == 0:
                        state["osb"] = outp.tile([P, 4, QT], BF16, name="osb")
                    osb = state["osb"]
                    if oc % 2 == 0:
                        nc.scalar.copy(osb[:, oc % 4, :], acc[:])
                    else:
                        nc.vector.tensor_copy(osb[:, oc % 4, :], acc[:])
                    if oc % 4 == 3:
                        nc.sync.dma_start(
                            out=o_r[:, oc - 3:oc + 1,
                                    st * QT:(st + 1) * QT],
                            in_=osb[:])
                return f
            return [piece(oc) for oc in range(NKC)]

        class Feeder:
            """Doles out independent PE micro-steps to hide ACT latency."""
            def __init__(self):
                self.chunks = []

            def add(self, chunks):
                self.chunks.extend(chunks)

            def step(self, n):
                for _ in range(n):
                    if self.chunks:
                        self.chunks.pop(0)()

            def drain(self):
                self.step(len(self.chunks))

        def emit_attn(h, t, pools, feeder, per_pair):
            s_ps, o_ps, l_ps, thp, pp, np_ = pools
            o_acc = o_ps.tile([P, QT], F32, name="o_acc")
            l_acc = l_ps.tile([1, QT], F32, name="l_acc")
            npair = 2 * t + 2
            q_ap = qT[h][:, t * QT:(t + 1) * QT]

            def emit_pv(pT, p, last):
                for i in range(2):
                    kc = 2 * p + i
                    nc.tensor.matmul(
                        o_acc[:],
                        v_sb[h][:, kc * P:(kc + 1) * P],
                        pT[:, i * QT:(i + 1) * QT],
                        start=(kc == 0), stop=(last and i == 1),
                    )
                    nc.tensor.matmul(
                        l_acc[:], ones_bf[:, 0:1],
                        pT[:, i * QT:(i + 1) * QT],
                        start=(kc == 0), stop=(last and i == 1),
                    )

            prev = None
            for p in range(npair):
                sp = s_ps.tile([P, 2 * QT], F32, name="sp")
                for i in range(2):
                    kc = 2 * p + i
                    nc.tensor.matmul(
                        sp[:, i * QT:(i + 1) * QT],
                        kT[h][:, kc * P:(kc + 1) * P], q_ap,
                        start=True, stop=True,
                    )
                feeder.step(per_pair)
                th = thp.tile([P, 2 * QT], F32, name="th")
                nc.scalar.activation(th[:], sp[:], Tanh, scale=C1)
                pT = pp.tile([P, 2 * QT], BF16, name="pTt")
                nc.scalar.activation(pT[:], th[:], Exp, scale=SOFTCAP)
                # masked pairs are the last two: p==2t (u=0,1), p==2t+1 (u=2,3)
                u0 = 2 * (p - 2 * t)
                if u0 >= 0:
                    nc.vector.tensor_mul(
                        pT[:], pT[:], mask_sb[:, u0 * QT:(u0 + 2) * QT])
                if prev is not None:
                    emit_pv(prev[0], prev[1], last=False)
                prev = (pT, p)
            emit_pv(prev[0], prev[1], last=True)
            recip = np_.tile([1, QT], F32, name="recip")
            nc.vector.reciprocal(recip[:], l_acc[:])
            bcast = np_.tile([P, QT], F32, name="bcast")
            nc.gpsimd.partition_broadcast(bcast[:], recip[:])
            nc.vector.tensor_mul(
                oT[h][:, t * QT:(t + 1) * QT], o_acc[:], bcast[:])

        # ---------- phase A (pre-attention part) ----------
        # head 0's q/k + the first 4 v chunks. Tiles needing only the x
        # first halves come first, k-interleaved within 3-tile windows so
        # the PE tracks the DMA wavefront instead of stalling on one tile.
        def interleave(units):
            out = []
            for step in range(max(len(u) for u in units)):
                for u in units:
                    if step < len(u):
                        out.append(u[step])
            return out

        # The A phase is DMA-bound (~35us of input wavefront), so all v
        # chunks ride along in its PE bubbles, ordered by which x quarter
        # they need.
        with ExitStack() as ctxA:
            qkA = ctxA.enter_context(
                tc.tile_pool(name="qkA", bufs=4, space="PSUM"))
            pre = Feeder()
            pre.add(interleave([qk_chunks(qkA, 0, 0), qk_chunks(qkA, 2, 0)]))
            pre.add(interleave([qk_chunks(qkA, 0, 1), qk_chunks(qkA, 2, 1)]))
            for sc in range(0, 8):
                pre.add(v_chunks(qkA, sc))
            pre.add(interleave([qk_chunks(qkA, 0, 2), qk_chunks(qkA, 2, 2)]))
            for sc in range(8, 12):
                pre.add(v_chunks(qkA, sc))
            pre.add(interleave([qk_chunks(qkA, 0, 3), qk_chunks(qkA, 2, 3)]))
            for sc in range(12, 16):
                pre.add(v_chunks(qkA, sc))
            pre.drain()

        # ---------- phase B0: head-0 attention + A-fill ----------
        s_ps = ctx.enter_context(tc.tile_pool(name="s_ps", bufs=2, space="PSUM"))
        o_ps = ctx.enter_context(tc.tile_pool(name="o_ps", bufs=1, space="PSUM"))
        l_ps = ctx.enter_context(tc.tile_pool(name="l_ps", bufs=1, space="PSUM"))
        thp = ctx.enter_context(tc.tile_pool(name="tanh", bufs=3))
        pp = ctx.enter_context(tc.tile_pool(name="pT", bufs=3))
        np_ = ctx.enter_context(tc.tile_pool(name="norm", bufs=2))
        bpools = (s_ps, o_ps, l_ps, thp, pp, np_)

        # shared fill/output-projection PSUM pool (one tag, 2 banks)
        fps = ctx.enter_context(tc.tile_pool(name="fps", bufs=2, space="PSUM"))
        outp = ctx.enter_context(tc.tile_pool(name="out", bufs=4))

        fill = Feeder()
        for st in (0, 1):
            fill.add(qk_chunks(fps, 1, st))
            fill.add(qk_chunks(fps, 3, st))
        for t in range(NQT):
            emit_attn(0, t, bpools, fill, per_pair=1)
        # q1/k1 st2/st3 are first needed by B1 t2/t3: defer them into the
        # otherwise-unfilled B1 t0/t1 windows.
        for st in (2, 3):
            fill.add(qk_chunks(fps, 1, st))
            fill.add(qk_chunks(fps, 3, st))

        # ---------- phase B1 + C: head-1 attention + output projection ----
        for t in range(NQT):
            emit_attn(1, t, bpools, fill, per_pair=4)
            fill.add(c_chunks(t, outp, fps))
        fill.drain()


_NC_CACHE = None


def _get_nc():
    global _NC_CACHE
    if _NC_CACHE is None:
        _NC_CACHE = build_nc()
    return _NC_CACHE


def _rope_perm():
    """per-head column permutation de-interleaving rotary pairs"""
    perm = np.zeros(DM, np.int64)
    for h in range(H):
        base = h * HD
        perm[base:base + HD // 2] = base + np.arange(0, HD, 2)
        perm[base + HD // 2:base + HD] = base + np.arange(1, HD, 2)
    return perm


def make_in_maps(x, wq, wk, wv, wo, freqs_cos, freqs_sin):
    x = np.asarray(x, np.float32).reshape(S, DM)
    wq = np.asarray(wq, np.float32)
    wk = np.asarray(wk, np.float32)
    wv = np.asarray(wv, np.float32)
    wo = np.asarray(wo, np.float32)
    xT = np.ascontiguousarray(x.T).astype(NPBF16)
    perm = _rope_perm()
    wq_p = wq[:, perm]
    wk_p = wk[:, perm]
    # transposed rope tables: C = [cosT; cosT], S' = [-sinT; sinT]
    cosT = np.asarray(freqs_cos, np.float32).T  # [64, S]
    sinT = np.asarray(freqs_sin, np.float32).T
    cosT2 = np.concatenate([cosT, cosT], axis=0).astype(NPBF16)
    sinT2 = np.concatenate([-sinT, sinT], axis=0).astype(NPBF16)
    # mask[i, u*QT + j] = 1 if i <= j - 128*u else 0  (keep kj <= qi)
    i_idx = np.arange(P)[:, None]
    j_idx = np.arange(QT)[None, :]
    mask = np.concatenate(
        [(i_idx <= j_idx - P * u) for u in range(4)], axis=1
    ).astype(NPBF16)
    in_maps = []
    for c in range(N_CORES):
        cs = slice(c * CW, (c + 1) * CW)
        h0 = slice(c * CW, c * CW + HD)
        h1 = slice(c * CW + HD, (c + 1) * CW)
        # device column order: [q-h0, k-h0, v, q-h1, k-h1]
        w_all = np.concatenate(
            [wq_p[:, h0], wk_p[:, h0], wv[:, cs],
             wq_p[:, h1], wk_p[:, h1]], axis=1).astype(NPBF16)
        wo_c = np.ascontiguousarray(wo[cs, :]).astype(NPBF16)
        in_maps.append({
            "xT": xT,
            "w_all": np.ascontiguousarray(w_all),
            "wo_c": wo_c,
            "cosT2": cosT2,
            "sinT2": sinT2,
            "mask": mask,
        })
    return in_maps


def assemble_output(results):
    acc = results[0]["outT"].astype(np.float32)
    for r in results[1:]:
        acc += np.asarray(r["outT"]).astype(np.float32)
    return np.ascontiguousarray(acc.T).reshape(1, S, DM).astype(np.float32)


def kernel(x, wq, wk, wv, wo, freqs_cos, freqs_sin):
    nc = _get_nc()
    in_maps = make_in_maps(x, wq, wk, wv, wo, freqs_cos, freqs_sin)
    res = run_bass_kernel_spmd(nc, in_maps, core_ids=list(range(N_CORES)))
    return assemble_output(res.results)


if __name__ == "__main__":
    rng = np.random.default_rng(0)
    ins = {
        "x": rng.standard_normal((1, S, DM), np.float32),
        "wq": rng.standard_normal((DM, DM), np.float32) / np.sqrt(DM),
        "wk": rng.standard_normal((DM, DM), np.float32) / np.sqrt(DM),
        "wv": rng.standard_normal((DM, DM), np.float32) / np.sqrt(DM),
        "wo": rng.standard_normal((DM, DM), np.float32) / np.sqrt(DM),
        "freqs_cos": rng.standard_normal((S, HD // 2), np.float32),
        "freqs_sin": rng.standard_normal((S, HD // 2), np.float32),
    }
    out = kernel(**ins)
    print("out", out.shape, out.dtype, np.abs(out).mean())


# revision 38
# speedup vs baseline: 1.2747x; 1.0220x over previous
"""Trainium2 Bass kernel for Llama-like attention (16 heads, tanh softcap, RoPE).

Sharding: tensor-parallel over heads, fully collective-free. Each of the 8
cores computes 2 heads end-to-end and a *partial* output projection
(o_local @ wo_rows_local)^T; the host sums the 8 partial outputs. With no
on-device collective, each core's NEFF span is pure local compute — no
cross-core rendezvous.

Per-core pipeline (engine-balanced against the ~165us PE floor):
  - q^T/k^T computed directly in transposed layout ([head_dim, s]) via
    matmul(w_slice^T, x^T): no PE transposes. Weight columns of wq/wk are
    pre-permuted on the host to de-interleave even/odd rotary pairs (the
    permutation cancels inside q.k).
  - RoPE in transposed layout straight out of PSUM: rot = A*C + swap(A)*S'
    with C = [cosT; cosT], S' = [-sinT; sinT]. The partition-half swap is
    done by two half-height Pool multiplies reading PSUM at a partition
    offset; the straight multiply and the final add run on Vector (bf16
    2x mode). ACT stays free for the softmax chain.
  - attention with scores transposed ([kj, qi]) so softmaxed probabilities
    feed the PV matmul directly as the moving operand. tanh softcap bounds
    scores, so softmax needs no row-max pass: p = exp(50*tanh(.)),
    l = ones-row matmul, o = p@v / l. Head 0's ACT-bound window is filled
    with head 1's q/k projection and the tail v chunks; head 1's windows
    are filled with the output-projection pieces for the q-tile that just
    finished.
  - output projection pieces acc[oc(128), st(512)] += wo_h[:, oc]^T @ oT_h
    accumulated over the 2 local heads, copied to SBUF bf16 (ACT/DVE
    alternating) and DMA'd per piece. Host sums partials and transposes.
"""

import os
import sys

for _p in ("/root/.axon_site/_ro/trn_rl_repo", "/opt/trn_rl_repo"):
    if os.path.isdir(_p) and _p not in sys.path:
        sys.path.append(_p)

import numpy as np
import ml_dtypes
from contextlib import ExitStack

import concourse.bass as bass
import concourse.bacc as bacc
import concourse.mybir as mybir
import concourse.tile as tile
from concourse.bass_utils import run_bass_kernel_spmd

BF16 = mybir.dt.bfloat16
F32 = mybir.dt.float32
NPBF16 = ml_dtypes.bfloat16

N_CORES = 8
S = 2048          # sequence length
DM = 2048         # model dim
H = 16            # heads
HD = 128          # head dim
HPC = H // N_CORES  # heads per core = 2
CW = HPC * HD     # per-core projection width = 256
P = 128
HW = HD // 2      # 64
QT = 512          # query tile (free dim of attention matmuls)
NQT = S // QT     # 4 query tiles per head
NSC = S // P      # 16 sequence chunks
NKC = DM // P     # 16 contraction chunks
NST = S // QT     # 4 s-tiles
SOFTCAP = 50.0
C1 = 1.0 / (SOFTCAP * np.sqrt(HD))

Tanh = mybir.ActivationFunctionType.Tanh
Exp = mybir.ActivationFunctionType.Exp


def build_nc(reps=1, single=False):
    nc = bacc.Bacc("TRN2", target_bir_lowering=False, num_devices=N_CORES)

    xT_d = nc.dram_tensor("xT", [DM, S], BF16, kind="ExternalInput")
    w_d = nc.dram_tensor("w_all", [DM, 3 * CW], BF16, kind="ExternalInput")
    wo_d = nc.dram_tensor("wo_c", [CW, DM], BF16, kind="ExternalInput")
    cos_d = nc.dram_tensor("cosT2", [P, S], BF16, kind="ExternalInput")
    sin_d = nc.dram_tensor("sinT2", [P, S], BF16, kind="ExternalInput")
    mask_d = nc.dram_tensor("mask", [P, 4 * QT], BF16, kind="ExternalInput")
    out_d = nc.dram_tensor("outT", [DM, S], BF16, kind="ExternalOutput")

    with tile.TileContext(nc) as tc:
        for _rep in range(reps):
            _emit_body(nc, tc, xT_d, w_d, wo_d, cos_d, sin_d, mask_d, out_d)
    nc.compile()
    return nc


def _emit_body(nc, tc, xT_d, w_d, wo_d, cos_d, sin_d, mask_d, out_d):
    with ExitStack() as ctx:
        # ---------- persistent SBUF ----------
        persist = ctx.enter_context(tc.tile_pool(name="persist", bufs=1))
        qT = [persist.tile([P, S], BF16, name=f"qT{h}") for h in range(HPC)]
        kT = [persist.tile([P, S], BF16, name=f"kT{h}") for h in range(HPC)]
        v_sb = [persist.tile([P, S], BF16, name=f"v{h}") for h in range(HPC)]
        oT = [persist.tile([P, S], BF16, name=f"oT{h}") for h in range(HPC)]
        mask_sb = persist.tile([P, 4 * QT], BF16, name="mask")
        ones_bf = persist.tile([P, 1], BF16, name="ones")
        cos_sb = persist.tile([P, S], BF16, name="cosT2")
        sin_sb = persist.tile([P, S], BF16, name="sinT2")
        wo_sb = [persist.tile([P, DM], BF16, name=f"wo{h}") for h in range(HPC)]
        xp = ctx.enter_context(tc.tile_pool(name="xT", bufs=1))
        wp = ctx.enter_context(tc.tile_pool(name="w", bufs=1))
        rp = ctx.enter_context(tc.tile_pool(name="rope", bufs=4))

        # DMA priority order (HWDGE + the transfer engines serialize, so
        # issue order IS arrival order): rope tables, then the pre-phase
        # wavefront (w cols [q0|k0|v] + x first halves, k-interleaved),
        # then x second halves, mask, w cols [q1|k1], wo. w_all columns
        # are host-reordered to [q0, k0, v, q1, k1] to enable the split.
        # Batched loads: HWDGE issue bandwidth (~0.63us per DMA) is the
        # startup bottleneck, so w/x load as 4-k-group DMAs via 3-D tiles
        # and partition-inner DRAM views; x additionally splits into
        # column halves so the first s-tiles unblock early.
        wt_all = wp.tile([P, NKC, 3 * CW], BF16, name="wt")
        xt_all = xp.tile([P, NKC, S], BF16, name="xt")
        w_r = w_d.rearrange("(k p) c -> p k c", p=P)
        x_r = xT_d.rearrange("(k p) c -> p k c", p=P)
        HS = S // 2
        for g in range(0, NKC, 4):
            nc.sync.dma_start(
                out=wt_all[:, g:g + 4, :], in_=w_r[:, g:g + 4, :])
            nc.scalar.dma_start(
                out=xt_all[:, g:g + 4, 0:HS], in_=x_r[:, g:g + 4, 0:HS])
            if g == 0:
                # rope tables: needed ~10us in, after the first k-group
                nc.scalar.dma_start(out=cos_sb[:], in_=cos_d[:])
                nc.scalar.dma_start(out=sin_sb[:], in_=sin_d[:])
        for g in range(0, NKC, 4):
            (nc.sync if g % 8 == 0 else nc.scalar).dma_start(
                out=xt_all[:, g:g + 4, HS:S], in_=x_r[:, g:g + 4, HS:S])
        nc.sync.dma_start(out=mask_sb[:], in_=mask_d[:])
        nc.vector.memset(ones_bf[:], 1.0)
        for h in range(HPC):
            nc.sync.dma_start(out=wo_sb[h][:], in_=wo_d[h * P:(h + 1) * P, :])

        # w_all column offsets after host reorder [q0, k0, v, q1, k1]
        W_OFF = {0: 0, 2: P, 1: 2 * P + CW, 3: 3 * P + CW}
        V_OFF = 2 * P

        def qk_chunks(pool, c, st):
            """q/k feature chunk c (0: q-h0, 1: q-h1, 2: k-h0, 3: k-h1),
            s-tile st, transposed layout + fused rope, as 4 PE micro-steps."""
            dst = (qT, kT)[c // HPC][c % HPC]
            wo_ = W_OFF[c]
            state = {}

            def mm(k0):
                def f():
                    if k0 == 0:
                        state["ps"] = pool.tile([P, QT], F32, name="f")
                    ps = state["ps"]
                    for k in range(k0, k0 + 4):
                        nc.tensor.matmul(
                            ps[:],
                            wt_all[:, k, wo_:wo_ + P],
                            xt_all[:, k, st * QT:(st + 1) * QT],
                            start=(k == 0), stop=(k == NKC - 1),
                        )
                    if k0 == NKC - 4:
                        ps = state["ps"]
                        cs = slice(st * QT, (st + 1) * QT)
                        # Pool cannot read PSUM: both rope multiplies run
                        # on DVE; the all-SBUF add goes to Pool.
                        t1 = rp.tile([P, QT], BF16, name="t1")
                        nc.vector.tensor_mul(t1[:], ps[:], cos_sb[:, cs])
                        t2 = rp.tile([P, QT], BF16, name="t2")
                        nc.vector.tensor_mul(
                            t2[0:HW, :], ps[HW:HD, :], sin_sb[0:HW, cs])
                        nc.vector.tensor_mul(
                            t2[HW:HD, :], ps[0:HW, :], sin_sb[HW:HD, cs])
                        nc.gpsimd.tensor_add(dst[:, cs], t1[:], t2[:])
                return f
            return [mm(k0) for k0 in range(0, NKC, 4)]

        def v_chunks(pool, sc):
            """v s-chunk sc in natural layout, as 2 PE micro-steps."""
            state = {}

            def mm(k0):
                def f():
                    if k0 == 0:
                        state["ps"] = pool.tile([P, QT], F32, name="f")
                    ps = state["ps"]
                    for k in range(k0, k0 + 8):
                        nc.tensor.matmul(
                            ps[:, 0:CW],
                            xt_all[:, k, sc * P:(sc + 1) * P],
                            wt_all[:, k, V_OFF:V_OFF + CW],
                            start=(k == 0), stop=(k == NKC - 1),
                        )
                    if k0 == NKC - 8:
                        for h in range(HPC):
                            nc.vector.tensor_copy(
                                v_sb[h][:, sc * P:(sc + 1) * P],
                                ps[:, h * HD:(h + 1) * HD],
                            )
                return f
            return [mm(0), mm(8)]

        o_r = out_d.rearrange("(o p) s -> p o s", p=P)

        def c_chunks(st, outp, c_ps, alt=False):
            """output-projection pieces for s-tile st, 1 PE micro-step each;
            results stage into 4-oc-wide tiles DMA'd as one transfer."""
            state = {}

            def piece(oc):
                def f():
                    acc = c_ps.tile([P, QT], F32, name="f")
                    for h in range(HPC):
                        nc.tensor.matmul(
                            acc[:],
                            wo_sb[h][:, oc * P:(oc + 1) * P],
                            oT[h][:, st * QT:(st + 1) * QT],
                            start=(h == 0), stop=(h == HPC - 1),
                        )
                    if oc % 4 == 0:
                        state["osb"] = outp.tile([P, 4, QT], BF16, name="osb")
                    osb = state["osb"]
                    # ACT carries the B1 tanh/exp chain: only 1 in 4 copies
                    # goes there — except in the drain round (alt), where
                    # ACT is free and copies alternate 50/50
                    if (oc % 2 == 0) if alt else (oc % 4 == 0):
                        nc.scalar.copy(osb[:, oc % 4, :], acc[:])
                    else:
                        nc.vector.tensor_copy(osb[:, oc % 4, :], acc[:])
                    if oc % 4 == 3:
                        nc.sync.dma_start(
                            out=o_r[:, oc - 3:oc + 1,
                                    st * QT:(st + 1) * QT],
                            in_=osb[:])
                return f
            return [piece(oc) for oc in range(NKC)]

        class Feeder:
            """Doles out independent PE micro-steps to hide ACT latency."""
            def __init__(self):
                self.chunks = []

            def add(self, chunks):
                self.chunks.extend(chunks)

            def step(self, n):
                for _ in range(n):
                    if self.chunks:
                        self.chunks.pop(0)()

            def drain(self):
                self.step(len(self.chunks))

        def emit_attn(h, t, pools, feeder, per_pair):
            s_ps, o_ps, l_ps, thp, pp, np_ = pools
            o_acc = o_ps.tile([P, QT], F32, name="o_acc")
            l_acc = l_ps.tile([1, QT], F32, name="l_acc")
            npair = 2 * t + 2
            q_ap = qT[h][:, t * QT:(t + 1) * QT]

            def emit_pv(pT, p, last):
                for i in range(2):
                    kc = 2 * p + i
                    nc.tensor.matmul(
                        o_acc[:],
                        v_sb[h][:, kc * P:(kc + 1) * P],
                        pT[:, i * QT:(i + 1) * QT],
                        start=(kc == 0), stop=(last and i == 1),
                    )
                    nc.tensor.matmul(
                        l_acc[:], ones_bf[:, 0:1],
                        pT[:, i * QT:(i + 1) * QT],
                        start=(kc == 0), stop=(last and i == 1),
                    )

            prev = None
            for p in range(npair):
                sp = s_ps.tile([P, 2 * QT], F32, name="sp")
                for i in range(2):
                    kc = 2 * p + i
                    nc.tensor.matmul(
                        sp[:, i * QT:(i + 1) * QT],
                        kT[h][:, kc * P:(kc + 1) * P], q_ap,
                        start=True, stop=True,
                    )
                feeder.step(per_pair)
                th = thp.tile([P, 2 * QT], F32, name="th")
                nc.scalar.activation(th[:], sp[:], Tanh, scale=C1)
                pT = pp.tile([P, 2 * QT], BF16, name="pTt")
                nc.scalar.activation(pT[:], th[:], Exp, scale=SOFTCAP)
                # masked pairs are the last two: p==2t (u=0,1), p==2t+1 (u=2,3)
                u0 = 2 * (p - 2 * t)
                if u0 >= 0:
                    nc.vector.tensor_mul(
                        pT[:], pT[:], mask_sb[:, u0 * QT:(u0 + 2) * QT])
                if prev is not None:
                    emit_pv(prev[0], prev[1], last=False)
                prev = (pT, p)
            emit_pv(prev[0], prev[1], last=True)
            recip = np_.tile([1, QT], F32, name="recip")
            nc.vector.reciprocal(recip[:], l_acc[:])
            bcast = np_.tile([P, QT], F32, name="bcast")
            nc.gpsimd.partition_broadcast(bcast[:], recip[:])
            nc.vector.tensor_mul(
                oT[h][:, t * QT:(t + 1) * QT], o_acc[:], bcast[:])

        # ---------- phase A (pre-attention part) ----------
        # head 0's q/k + the first 4 v chunks. Tiles needing only the x
        # first halves come first, k-interleaved within 3-tile windows so
        # the PE tracks the DMA wavefront instead of stalling on one tile.
        def interleave(units):
            out = []
            for step in range(max(len(u) for u in units)):
                for u in units:
                    if step < len(u):
                        out.append(u[step])
            return out

        # The A phase is DMA-bound (~35us of input wavefront), so all v
        # chunks ride along in its PE bubbles, ordered by which x quarter
        # they need.
        with ExitStack() as ctxA:
            qkA = ctxA.enter_context(
                tc.tile_pool(name="qkA", bufs=4, space="PSUM"))
            pre = Feeder()
            pre.add(interleave([qk_chunks(qkA, 0, 0), qk_chunks(qkA, 2, 0)]))
            pre.add(interleave([qk_chunks(qkA, 0, 1), qk_chunks(qkA, 2, 1)]))
            for sc in range(0, 8):
                pre.add(v_chunks(qkA, sc))
            pre.add(interleave([qk_chunks(qkA, 0, 2), qk_chunks(qkA, 2, 2)]))
            for sc in range(8, 12):
                pre.add(v_chunks(qkA, sc))
            pre.add(interleave([qk_chunks(qkA, 0, 3), qk_chunks(qkA, 2, 3)]))
            for sc in range(12, 16):
                pre.add(v_chunks(qkA, sc))
            pre.drain()

        # ---------- phase B0: head-0 attention + A-fill ----------
        # shared fill/output-projection PSUM pool (one tag, 2 banks);
        # created below the B pools so those can close before the drain
        fps = ctx.enter_context(tc.tile_pool(name="fps", bufs=2, space="PSUM"))
        outp = ctx.enter_context(tc.tile_pool(name="out", bufs=4))
        fill = Feeder()
        with ExitStack() as ctxB:
            s_ps = ctxB.enter_context(
                tc.tile_pool(name="s_ps", bufs=2, space="PSUM"))
            o_ps = ctxB.enter_context(
                tc.tile_pool(name="o_ps", bufs=1, space="PSUM"))
            l_ps = ctxB.enter_context(
                tc.tile_pool(name="l_ps", bufs=1, space="PSUM"))
            thp = ctxB.enter_context(tc.tile_pool(name="tanh", bufs=3))
            pp = ctxB.enter_context(tc.tile_pool(name="pT", bufs=3))
            np_ = ctxB.enter_context(tc.tile_pool(name="norm", bufs=2))
            bpools = (s_ps, o_ps, l_ps, thp, pp, np_)

            for st in (0, 1):
                fill.add(qk_chunks(fps, 1, st))
                fill.add(qk_chunks(fps, 3, st))
            for t in range(NQT):
                emit_attn(0, t, bpools, fill, per_pair=1)
            # q1/k1 st2/st3 are first needed by B1 t2/t3: defer them into
            # the otherwise-unfilled B1 t0/t1 windows.
            for st in (2, 3):
                fill.add(qk_chunks(fps, 1, st))
                fill.add(qk_chunks(fps, 3, st))

            # ---------- phase B1 + C: attention + output projection ----
            for t in range(NQT):
                emit_attn(1, t, bpools, fill, per_pair=4)
                if t < NQT - 1:
                    fill.add(c_chunks(t, outp, fps))
        # drain round: B pools are closed, give the last output-projection
        # round a wide PSUM pool so its pieces pipeline
        cD = ctx.enter_context(tc.tile_pool(name="cD", bufs=5, space="PSUM"))
        fill.add(c_chunks(NQT - 1, outp, cD, alt=True))
        fill.drain()


_NC_CACHE = None


def _get_nc():
    global _NC_CACHE
    if _NC_CACHE is None:
        _NC_CACHE = build_nc()
    return _NC_CACHE


def _rope_perm():
    """per-head column permutation de-interleaving rotary pairs"""
    perm = np.zeros(DM, np.int64)
    for h in range(H):
        base = h * HD
        perm[base:base + HD // 2] = base + np.arange(0, HD, 2)
        perm[base + HD // 2:base + HD] = base + np.arange(1, HD, 2)
    return perm


def make_in_maps(x, wq, wk, wv, wo, freqs_cos, freqs_sin):
    x = np.asarray(x, np.float32).reshape(S, DM)
    wq = np.asarray(wq, np.float32)
    wk = np.asarray(wk, np.float32)
    wv = np.asarray(wv, np.float32)
    wo = np.asarray(wo, np.float32)
    xT = np.ascontiguousarray(x.T).astype(NPBF16)
    perm = _rope_perm()
    wq_p = wq[:, perm]
    wk_p = wk[:, perm]
    # transposed rope tables: C = [cosT; cosT], S' = [-sinT; sinT]
    cosT = np.asarray(freqs_cos, np.float32).T  # [64, S]
    sinT = np.asarray(freqs_sin, np.float32).T
    cosT2 = np.concatenate([cosT, cosT], axis=0).astype(NPBF16)
    sinT2 = np.concatenate([-sinT, sinT], axis=0).astype(NPBF16)
    # mask[i, u*QT + j] = 1 if i <= j - 128*u else 0  (keep kj <= qi)
    i_idx = np.arange(P)[:, None]
    j_idx = np.arange(QT)[None, :]
    mask = np.concatenate(
        [(i_idx <= j_idx - P * u) for u in range(4)], axis=1
    ).astype(NPBF16)
    in_maps = []
    for c in range(N_CORES):
        cs = slice(c * CW, (c + 1) * CW)
        h0 = slice(c * CW, c * CW + HD)
        h1 = slice(c * CW + HD, (c + 1) * CW)
        # device column order: [q-h0, k-h0, v, q-h1, k-h1]
        w_all = np.concatenate(
            [wq_p[:, h0], wk_p[:, h0], wv[:, cs],
             wq_p[:, h1], wk_p[:, h1]], axis=1).astype(NPBF16)
        wo_c = np.ascontiguousarray(wo[cs, :]).astype(NPBF16)
        in_maps.append({
            "xT": xT,
            "w_all": np.ascontiguousarray(w_all),
            "wo_c": wo_c,
            "cosT2": cosT2,
            "sinT2": sinT2,
            "mask": mask,
        })
    return in_maps


def assemble_output(results):
    acc = results[0]["outT"].astype(np.float32)
    for r in results[1:]:
        acc += np.asarray(r["outT"]).astype(np.float32)
    return np.ascontiguousarray(acc.T).reshape(1, S, DM).astype(np.float32)


def kernel(x, wq, wk, wv, wo, freqs_cos, freqs_sin):
    nc = _get_nc()
    in_maps = make_in_maps(x, wq, wk, wv, wo, freqs_cos, freqs_sin)
    res = run_bass_kernel_spmd(nc, in_maps, core_ids=list(range(N_CORES)))
    return assemble_output(res.results)


if __name__ == "__main__":
    rng = np.random.default_rng(0)
    ins = {
        "x": rng.standard_normal((1, S, DM), np.float32),
        "wq": rng.standard_normal((DM, DM), np.float32) / np.sqrt(DM),
        "wk": rng.standard_normal((DM, DM), np.float32) / np.sqrt(DM),
        "wv": rng.standard_normal((DM, DM), np.float32) / np.sqrt(DM),
        "wo": rng.standard_normal((DM, DM), np.float32) / np.sqrt(DM),
        "freqs_cos": rng.standard_normal((S, HD // 2), np.float32),
        "freqs_sin": rng.standard_normal((S, HD // 2), np.float32),
    }
    out = kernel(**ins)
    print("out", out.shape, out.dtype, np.abs(out).mean())
